# revision 1
# baseline (speedup 1.0000x reference)
"""Data-parallel attention kernel for 8 Trainium2 NeuronCores.

Reference computation (per sample b):
  q = w1 . x (channel contraction) + b1      [1,H,W]
  k = w2 . x + b2                            [1,H,W]
  v = w3 . x + b3                            [C,H,W]
  scores[i,j] = sum_w q[i,w] k[j,w]          [H,H]
  attn = softmax(scores, axis=-1)
  out[c,i,w] = sum_j attn[i,j] v[c,j,w]      [C,H,W]

Sharding: batch B=64 split 8 ways (8 samples per core), weights replicated.
Each sample's attention map is independent -> no cross-core communication.
"""
import numpy as np
import jax
import jax.numpy as jnp
from functools import partial

B, C, H, W = 64, 8, 256, 256
N_CORES = 8

_kernel_fn = None


def _local_attn(x, w1, b1, w2, b2, w3, b3):
    # x: [Bl, C, H, W] local shard
    q = jnp.einsum('bchw,oc->bohw', x, w1) + b1[None, :, None, None]
    k = jnp.einsum('bchw,oc->bohw', x, w2) + b2[None, :, None, None]
    v = jnp.einsum('bchw,oc->bohw', x, w3) + b3[None, :, None, None]
    scores = jnp.einsum('bhw,bgw->bhg', q[:, 0], k[:, 0])
    attn = jax.nn.softmax(scores, axis=-1)
    out = jnp.einsum('bhg,bcgw->bchw', attn, v)
    return out


def _get_fn():
    global _kernel_fn
    if _kernel_fn is None:
        _kernel_fn = jax.pmap(
            _local_attn,
            in_axes=(0, None, None, None, None, None, None),
        )
    return _kernel_fn


def kernel(x, w1, b1, w2, b2, w3, b3):
    x = np.asarray(x, dtype=np.float32)
    xs = x.reshape(N_CORES, B // N_CORES, C, H, W)
    fn = _get_fn()
    out = fn(xs,
             np.asarray(w1, np.float32), np.asarray(b1, np.float32),
             np.asarray(w2, np.float32), np.asarray(b2, np.float32),
             np.asarray(w3, np.float32), np.asarray(b3, np.float32))
    out = np.asarray(out)
    return out.reshape(B, C, H, W)


# revision 2
# speedup vs baseline: 1.2023x; 1.2023x over previous
"""Data-parallel spatial-attention kernel for 8 Trainium2 NeuronCores.

Reference computation (per sample b):
  q = w1 . x (1x1 conv) + b1                 [1,H,W]
  k = w2 . x + b2                            [1,H,W]
  v = w3 . x + b3                            [C,H,W]
  scores[i,j] = sum_w q[i,w] k[j,w]          [H,H]
  attn = softmax(scores, axis=-1)
  out[c,i,w] = sum_j attn[i,j] v[c,j,w]      [C,H,W]

Sharding: batch B=64 split 8 ways (8 samples per core), weights replicated;
each sample's attention map is independent so no cross-core communication.
The three 1x1-conv projections are fused into one [C+2, C] weight so x is
read once instead of three times per core.
"""
import numpy as np
import jax
import jax.numpy as jnp

B, C, H, W = 64, 8, 256, 256
N_CORES = 8

_kernel_fn = None


def _local_attn(x, wall, ball):
    # wall: [C+2, C] rows stacked [w1; w2; w3]; ball: [C+2]
    qkv = jnp.einsum('bchw,oc->bohw', x, wall) + ball[None, :, None, None]
    q = qkv[:, 0]                # [Bl,H,W]
    k = qkv[:, 1]                # [Bl,H,W]
    v = qkv[:, 2:]               # [Bl,C,H,W]
    scores = jnp.einsum('bhw,bgw->bhg', q, k)
    attn = jax.nn.softmax(scores, axis=-1)
    out = jnp.einsum('bhg,bcgw->bchw', attn, v)
    return out


def _get_fn():
    global _kernel_fn
    if _kernel_fn is None:
        _kernel_fn = jax.pmap(_local_attn, in_axes=(0, None, None))
    return _kernel_fn


def kernel(x, w1, b1, w2, b2, w3, b3):
    x = np.asarray(x, dtype=np.float32)
    xs = x.reshape(N_CORES, B // N_CORES, C, H, W)
    wall = np.concatenate([np.asarray(w1, np.float32),
                           np.asarray(w2, np.float32),
                           np.asarray(w3, np.float32)], axis=0)
    ball = np.concatenate([np.asarray(b1, np.float32),
                           np.asarray(b2, np.float32),
                           np.asarray(b3, np.float32)], axis=0)
    out = _get_fn()(xs, wall, ball)
    return np.asarray(out).reshape(B, C, H, W)
